# revision 1
# baseline (speedup 1.0000x reference)
"""PoseConsistencyLoss Trainium2 kernel (8-core SPMD Bass/Tile).

Math: the reference's outputs (loss, num_matches, mean_distance) depend only on
the per-landmark min squared distance over all splats:
  - matched = splat_positions[argmin] makes sum(sqerr) == min_dist^2 exactly,
  - so loss = sum(valid*minsq)/max(3*num,1), mean = sum(valid*sqrt(minsq))/max(num,1),
    num = sum(minsq < 1.0).
Sharding: splats split across 8 cores (8192 each); each core computes partial
column-mins of the [8192 x 2048] distance matrix; host does the 8-way min and
the masked reduction over 2048 landmarks.

Device computes E[m,n] = -2 c_m . s_n + ||s_n||^2  (c = landmarks in camera
frame). ||c_m||^2 is a per-landmark (per-PSUM-row) constant, so it cannot
change the argmin over n -- the host adds it back after the cross-core min:
  minsq = max(min_cores min_n E + ||c||^2, 0).

E is ONE K=15 f32r matmul per [128 x 512] tile (PE cost is K-independent:
moving-free-size x 1 cy/row for f32r, vs 4 cy/row for fp32). f32r rounds
operands to 12-bit mantissa; full precision is recovered with hi/lo splits
folded into the K dim:
  k 0-2 : -2c_hi * s_hi      k 9-11 : 1 * (s^2)_hi
  k 3-5 : -2c_lo * s_hi      k 12-14: 1 * (s^2)_lo
  k 6-8 : -2c_hi * s_lo
(dropped lo*lo terms ~2^-24). Both feature tensors are built host-side
(layout/precision prep, ~0.1% of the FLOPs) so the PE can start as soon as
one DMA lands; the distance matrix + column-min stay on device.

Column-min consumes PSUM with an ACT/DVE split (DVE tensor_tensor_reduce
faults on this runtime; GPSIMD cannot read PSUM and only supports add/mult
tensor_tensor, so Pool is useless for min). Only the DVE can reduce along
the free axis (always 1 elem/lane/cy), but DVE tensor_tensor on pure-SBUF
bf16 runs in 2x_1p mode (2 elem/lane/cy). So: the first `group` spans are
copied PSUM->SBUF as bf16 by the Scalar engine via Identity(E + csq[m])
(the per-partition bias restores D^2 >= 0 so bf16's relative precision
lands on the small near-min distances), the DVE cascade-merges the bf16
copies (elementwise min, 2x) and min-reduces the result (1x); remaining
spans are min-reduced by DVE directly from PSUM in fp32 (csq added to
those column mins afterwards as a per-partition scalar).
"""

import os
import sys
import time

sys.path.insert(0, "/opt/trn_rl_repo")

import numpy as np

import concourse.bass as bass
import concourse.bacc as bacc
import concourse.tile as tile
from concourse import mybir
from concourse.bass_utils import run_bass_kernel_spmd

# Disk-cache NEFF compiles.
import concourse.bass_utils as _bu
import concourse.bass2jax as _b2j

_orig_compile_bir = _bu.compile_bir_kernel
_NEFF_CACHE = os.environ.get("BASS_NEFF_CACHE_DIR", "/tmp/bass_neff_cache")


def _cached_compile_bir(bir_json, tmpdir, neff_name="file.neff"):
    import hashlib
    import shutil

    h = hashlib.sha256(bir_json).hexdigest()[:24]
    os.makedirs(_NEFF_CACHE, exist_ok=True)
    cpath = os.path.join(_NEFF_CACHE, f"{h}_{neff_name}")
    out = os.path.join(tmpdir, neff_name)
    if os.path.exists(cpath):
        shutil.copyfile(cpath, out)
        return out
    p = _orig_compile_bir(bir_json, tmpdir, neff_name=neff_name)
    try:
        shutil.copyfile(p, cpath)
    except OSError:
        pass
    return p


_bu.compile_bir_kernel = _cached_compile_bir
_b2j.compile_bir_kernel = _cached_compile_bir

F32 = mybir.dt.float32
F32R = mybir.dt.float32r
BF16 = mybir.dt.bfloat16
I32 = mybir.dt.int32
AF = mybir.ActivationFunctionType
ALU = mybir.AluOpType
AX = mybir.AxisListType

HI_MASK = 0xFFFFF000  # keep sign+exp+11 mantissa bits (fp32r-exact)
BIG = 3.0e38

FULL_CFG = dict(
    n_cores=8,
    s_per_core=8192,   # splats per core
    m_total=2048,      # landmarks
    span=1024,         # psum span (free elems, 2 banks)
    psum_bufs=4,
    group=7,           # spans per mt routed via ACT bf16 copy + DVE 2x merge
)

KX = 15  # feature rows


def build(cfg):
    """Build the SPMD Bass program."""
    C = cfg["n_cores"]
    S = cfg["s_per_core"]
    M = cfg["m_total"]
    SPAN = cfg["span"]
    MMSZ = 512  # matmul moving free dim (hw max)
    assert SPAN % MMSZ == 0 and S % SPAN == 0 and M % 128 == 0
    MT = M // 128
    NSPAN = S // SPAN
    G = cfg["group"]
    assert 0 <= G <= NSPAN and G != 1

    nc = bacc.Bacc(
        "TRN2", target_bir_lowering=False, debug=False, num_devices=C
    )

    # ---- I/O ----
    featsp_d = nc.dram_tensor("featsp", [KX, S], F32R, kind="ExternalInput")
    featlm_d = nc.dram_tensor("featlm", [KX, M], F32R, kind="ExternalInput")
    csq_d = nc.dram_tensor("csq", [128, M // 128], F32, kind="ExternalInput")
    part_out_d = nc.dram_tensor("partial", [M], F32, kind="ExternalOutput")

    with tile.TileContext(nc) as tc:
        with (
            tc.tile_pool(name="persist", bufs=1) as persist,
            tc.tile_pool(name="setup", bufs=1) as setup,
            tc.tile_pool(name="stream", bufs=max(3, G + 1)) as stream,
            tc.tile_pool(name="mstream", bufs=3) as mstream,
        ):
            # ================= landmark features (host-built) =================
            feat_lm = persist.tile([KX, M], F32R)
            nc.sync.dma_start(feat_lm[:], featlm_d[:])
            csq = persist.tile([128, MT], F32)
            nc.sync.dma_start(csq[:], csq_d[:])

            # ================= splat features (host-built) =================
            # feat_sp rows: 0-2 s_hi, 3-5 s_hi(dup), 6-8 s_lo, 9-11 sq_hi,
            # 12-14 sq_lo. Chunked DMA so the first matmul only waits for
            # its own span's chunk.
            feat_sp = persist.tile([KX, S], F32R)
            for si in range(NSPAN):
                nc.sync.dma_start(
                    feat_sp[:, si * SPAN : (si + 1) * SPAN],
                    featsp_d[:, si * SPAN : (si + 1) * SPAN],
                )

            # ================= main loop =================
            pp = tc.alloc_tile_pool(name="psum", bufs=cfg["psum_bufs"], space="PSUM")
            minsq = persist.tile([128, MT], F32)
            # per-mt column mins accumulate into [128, MT] tiles; the csq add
            # for the direct (E-space) mins and the final combine run ONCE at
            # the end, keeping the steady-state DVE queue to 8 ops per mt.
            emins = persist.tile([128, MT], F32)
            emins2 = persist.tile([128, MT], F32)
            nc.vector.memset(emins2[:], BIG)
            bmins = persist.tile([128, MT], F32)
            for mt in range(MT):
                lhs = feat_lm[:, mt * 128 : (mt + 1) * 128]
                csq_mt = csq[:, mt : mt + 1]
                # ACT's 1126ns/span copy rate x7 exceeds PE's 7315ns/mt, so
                # alternate 7 and 6 ACT spans per mt (avg 6.5 matches PE);
                # the last mt ends on direct spans for a short tail.
                if mt == MT - 1:
                    directs = (NSPAN - 2, NSPAN - 1)
                elif mt % 2:
                    directs = (0, 1)
                else:
                    directs = (0,)
                m = None
                for si in range(NSPAN):
                    ps = pp.tile([128, SPAN], F32, tag="ps")
                    for h in range(SPAN // MMSZ):
                        off = si * SPAN + h * MMSZ
                        nc.tensor.matmul(
                            ps[:, h * MMSZ : (h + 1) * MMSZ],
                            lhs,
                            feat_sp[:, off : off + MMSZ],
                            start=True,
                            stop=True,
                        )
                    if si not in directs:
                        # D^2 = E + ||c||^2 in bf16; per-partition bias AP
                        sc = stream.tile([128, SPAN], BF16, tag="actcopy")
                        nc.scalar.activation(
                            sc[:], ps[:], AF.Identity, bias=csq_mt, scale=1.0
                        )
                        if m is None:
                            m = sc
                        else:
                            m2 = mstream.tile([128, SPAN], BF16, tag="merge")
                            nc.vector.tensor_tensor(m2[:], m[:], sc[:], ALU.min)
                            m = m2
                    else:
                        # direct span: PSUM buf freed by a single DVE op, not
                        # the whole merge cascade (E-space; csq added at end)
                        dst = emins if si == directs[0] else emins2
                        nc.vector.tensor_reduce(
                            dst[:, mt : mt + 1], ps[:], AX.X, ALU.min
                        )
                nc.vector.tensor_reduce(
                    bmins[:, mt : mt + 1], m[:], AX.X, ALU.min
                )
            # emins/emins2 hold min E = min(D^2) - csq; bmins bf16 min D^2
            edsq = setup.tile([128, MT], F32, tag="edsq")
            nc.vector.tensor_tensor(edsq[:], emins[:], emins2[:], ALU.min)
            edsq2 = setup.tile([128, MT], F32, tag="edsq2")
            nc.vector.tensor_add(edsq2[:], edsq[:], csq[:])
            nc.vector.tensor_tensor(minsq[:], edsq2[:], bmins[:], ALU.min)
            pp.release()

            # per-core partial min out; global min + masked loss on host
            nc.sync.dma_start(
                part_out_d[:].rearrange("(p f) -> p f", p=128), minsq[:]
            )

    nc.compile()
    return nc


def _f32r_trunc(x):
    return (np.ascontiguousarray(x, np.float32).view(np.uint32) & np.uint32(HI_MASK)).view(np.float32)


def _landmarks_cam(camera_pose, landmarks_3d):
    pose = np.asarray(camera_pose, np.float32)
    lm = np.asarray(landmarks_3d, np.float32)
    hom = np.concatenate([lm, np.ones((lm.shape[0], 1), np.float32)], axis=1)
    return (pose @ hom.T).T[:, :3].astype(np.float32)  # [M, 3]


def make_in_maps(cfg, splat_positions, camera_pose, landmarks_3d):
    C = cfg["n_cores"]
    S = cfg["s_per_core"]
    M = cfg["m_total"]
    MT = M // 128
    sp = np.ascontiguousarray(np.asarray(splat_positions, np.float32))
    cam = _landmarks_cam(camera_pose, landmarks_3d)  # [M, 3]
    m2c = (-2.0 * cam).astype(np.float32)
    hi = _f32r_trunc(m2c)
    lo = _f32r_trunc(m2c - hi)
    featlm = np.empty((KX, M), np.float32)
    featlm[0:3] = hi.T
    featlm[3:6] = lo.T
    featlm[6:9] = hi.T
    featlm[9:15] = 1.0
    csq = np.sum(cam ** 2, axis=1, dtype=np.float32)  # [M]
    # landmark m = mt*128 + p lives at minsq[p, mt]
    csq_aligned = np.ascontiguousarray(csq.reshape(MT, 128).T)  # [128, MT]
    maps = []
    for c in range(C):
        shard = sp[c * S : (c + 1) * S]  # [S, 3]
        sT = shard.T  # [3, S]
        s_hi = _f32r_trunc(sT)
        s_lo = _f32r_trunc(sT - s_hi)
        sq = (sT * sT).astype(np.float32)
        sq_hi = _f32r_trunc(sq)
        sq_lo = _f32r_trunc(sq - sq_hi)
        featsp = np.empty((KX, S), np.float32)
        featsp[0:3] = s_hi
        featsp[3:6] = s_hi
        featsp[6:9] = s_lo
        featsp[9:12] = sq_hi
        featsp[12:15] = sq_lo
        maps.append(
            {
                "featsp": featsp,
                "featlm": featlm,
                "csq": csq_aligned,
            }
        )
    return maps


_COMPILED = None


def _get_compiled():
    global _COMPILED
    if _COMPILED is None:
        _COMPILED = build(FULL_CFG)
    return _COMPILED


def kernel(
    splat_positions,
    camera_pose,
    landmarks_3d,
    landmarks_2d=None,
    camera_intrinsics=None,
    **_unused,
):
    nc = _get_compiled()
    in_maps = make_in_maps(FULL_CFG, splat_positions, camera_pose, landmarks_3d)
    core_ids = list(range(FULL_CFG["n_cores"]))
    try:
        res = run_bass_kernel_spmd(nc, in_maps, core_ids)
    except Exception:
        # one retry -- a previous run can leave the device wedged
        time.sleep(5.0)
        res = run_bass_kernel_spmd(nc, in_maps, core_ids)

    # host-side: cross-core min of per-core min-D^2, masked reduction
    parts = np.stack([r["partial"] for r in res.results], axis=0)  # [C, M]
    msq = np.maximum(parts.min(axis=0), np.float32(0.0)).astype(np.float32)
    d = np.sqrt(msq)
    valid = d < np.float32(1.0)
    num = np.int32(valid.sum())
    loss = np.float32(
        (msq * valid).sum(dtype=np.float32)
        / max(np.float32(3.0) * np.float32(num), np.float32(1.0))
    )
    meand = np.float32(
        (d * valid).sum(dtype=np.float32)
        / max(np.float32(num), np.float32(1.0))
    )
    return loss, num, meand


if __name__ == "__main__":
    # smoke-test build only
    build(FULL_CFG)
    print("build ok")

